# revision 22
# baseline (speedup 1.0000x reference)
"""Fused ParallelTransformerBlock kernel for 8 Trainium2 NeuronCores.

Sharding: Megatron-style tensor-parallel (2-way over heads + mlp_hidden)
x data-parallel (4-way over batch). Core c handles batch c//2 with
head/mlp shard c%2. Each core computes a partial output of linear2
(no residual, no bias); the host sums the two partials per batch and
adds x + b2 in fp32.

All tensors are bf16 except PSUM accumulation, norm stats and the final
output (fp32). All intermediates stay in SBUF. Every DRAM tensor is
pre-transposed on the host into per-partition-contiguous layout so each
DMA needs only one descriptor per partition (descriptor generation on
the DGE, not bandwidth, dominates scattered transfers). QK^T matmuls
for a head pair run row-tiled (base partitions 0/64) so the two K=64
matmuls overlap on the PE array.
"""
import numpy as np

import concourse.bass as bass
import concourse.tile as tile
from concourse import bacc, mybir

DIM = 1024
L = 2048
B = 4
H = 16
DH = 64
MLP = 3072
EPS_LN = 1e-6
EPS_RMS = 1e-6

P = 128
KD = DIM // P          # 8 k-tiles over model dim
TT = L // P            # 16 token tiles
HL = H // 2            # 8 heads per core
NP = HL // 2           # 4 head pairs
MLPL = MLP // 2        # 1536 mlp columns per core
FT = MLPL // P         # 12 mlp feature tiles
AKT = HL * DH // P     # 4 attn k-tiles into linear2
KT2 = AKT + FT         # 16 linear2 k-tiles

F32 = mybir.dt.float32
BF16 = mybir.dt.bfloat16
AF = mybir.ActivationFunctionType
ALU = mybir.AluOpType
AX = mybir.AxisListType


def gen_program(repeat: int = 1):
    nc = bacc.Bacc("TRN2", target_bir_lowering=False, debug=False, num_devices=8)

    # all inputs pre-transposed to [partition, ...contiguous...] on host
    xb = nc.dram_tensor("xb", (P, TT, DIM), BF16, kind="ExternalInput")
    pe2 = nc.dram_tensor("pe2", (P, 2, TT, 2, DH), BF16, kind="ExternalInput")
    w1qkv = nc.dram_tensor("w1qkv", (P, KD, 3 * HL * DH), BF16, kind="ExternalInput")
    w1mlp = nc.dram_tensor("w1mlp", (P, KD, MLPL), BF16, kind="ExternalInput")
    w2 = nc.dram_tensor("w2", (P, KT2, DIM), BF16, kind="ExternalInput")
    ident_in = nc.dram_tensor("ident", (P, P), BF16, kind="ExternalInput")
    y = nc.dram_tensor("y", (P, TT, DIM), F32, kind="ExternalOutput")

    from contextlib import ExitStack
    with tile.TileContext(nc) as tc, ExitStack() as es:
        pool_const = es.enter_context(tc.tile_pool(name="const", bufs=1))
        pool_w1024 = es.enter_context(tc.tile_pool(name="w1024", bufs=3))
        pool_w512 = es.enter_context(tc.tile_pool(name="w512", bufs=4))
        pool_small = es.enter_context(tc.tile_pool(name="small", bufs=8))
        pool_ps1024 = es.enter_context(tc.tile_pool(name="ps1024", bufs=2, space="PSUM"))
        pool_psb = es.enter_context(tc.tile_pool(name="psb", bufs=2, space="PSUM"))
        pool_pacc = es.enter_context(tc.tile_pool(name="ppacc", bufs=2, space="PSUM"))

        ident = pool_const.tile([P, P], BF16, tag="ident")
        nc.sync.dma_start(ident, ident_in[:, :])
        epsc = pool_const.tile([P, 1], F32, tag="epsc")
        nc.vector.memset(epsc, EPS_LN)
        ones_sb = pool_const.tile([P, DH], BF16, tag="ones_sb")
        nc.vector.memset(ones_sb, 1.0)

        def body():
            es_res = ExitStack()
            pool_res = es_res.enter_context(
                tc.tile_pool(name="pres", bufs=1, side="right"))
            # v laid out per (kt, head) as [tok-in-tile, kt, h, DH+1] with a
            # ones column at the end (accumulates the softmax denominator).
            v_sb = pool_res.tile([P, TT, HL, DH + 1], BF16, tag="v_sb")
            nc.vector.memset(v_sb[:, :, :, DH:DH + 1], 1.0)
            mlp_sb = pool_res.tile([P, FT, L], BF16, tag="mlp_sb")

            es_xT = ExitStack()
            pool_xT = es_xT.enter_context(tc.tile_pool(name="pxT", bufs=1))
            xT = pool_xT.tile([P, KD, L], BF16, tag="xT")
            w1m = pool_xT.tile([P, KD, MLPL], BF16, tag="w1m")
            nc.scalar.dma_start(w1m, w1mlp[:, :, :])

            es_w1 = ExitStack()
            pool_wq = es_w1.enter_context(tc.tile_pool(name="pwq", bufs=1))
            w1sb = pool_wq.tile([P, KD, 3 * HL * DH], BF16, tag="w1sb")
            nc.gpsimd.dma_start(w1sb, w1qkv[:, :, :])
            pe_sb = pool_wq.tile([P, 2, TT, 2, DH], BF16, tag="pe_sb")
            nc.scalar.dma_start(pe_sb, pe2[:, :, :, :, :])

            # ---- Phase A: LayerNorm + transpose to [dim, tok] ----
            es_xc = ExitStack()
            pool_xc = es_xc.enter_context(tc.tile_pool(name="pxc", bufs=2))
            for qt in range(4):
                xq = pool_xc.tile([P, 4, DIM], BF16, tag="xq")
                nc.sync.dma_start(xq, xb[:, 4 * qt:4 * qt + 4, :])
                for r in range(4):
                    tt = 4 * qt + r
                    ts = slice(tt * P, (tt + 1) * P)
                    xt = xq[:, r, :]
                    st = pool_small.tile([P, 2, 6], F32, tag="st")
                    nc.vector.bn_stats(st[:, 0, :], xt[:, 0:512])
                    nc.vector.bn_stats(st[:, 1, :], xt[:, 512:1024])
                    mv = pool_small.tile([P, 2], F32, tag="mv")
                    nc.vector.bn_aggr(mv, st)
                    std = pool_small.tile([P, 1], F32, tag="std")
                    nc.scalar.activation(std, mv[:, 1:2], AF.Sqrt, bias=epsc)
                    rstd = pool_small.tile([P, 1], F32, tag="rstd")
                    nc.vector.reciprocal(rstd, std)
                    nm = pool_small.tile([P, 1], F32, tag="nm")
                    nc.vector.tensor_scalar(
                        out=nm, in0=mv[:, 0:1], scalar1=rstd, scalar2=-1.0,
                        op0=ALU.mult, op1=ALU.mult)
                    xln = pool_w1024.tile([P, DIM], BF16, tag="w1024b")
                    nc.scalar.activation(xln, xt, AF.Identity,
                                         scale=rstd, bias=nm)
                    for g in range(2):
                        pst = pool_psb.tile([P, 512], BF16, tag="ps512b")
                        for j in range(4):
                            kd = g * 4 + j
                            nc.tensor.transpose(
                                pst[:, j * P:(j + 1) * P],
                                xln[:, kd * P:(kd + 1) * P], ident)
                        nc.vector.tensor_copy(
                            xT[:, g * 4:(g + 1) * 4, ts],
                            pst.rearrange("p (j t) -> p j t", j=4))
            es_xc.close()

            # ---- Phase B2: linear1 qkv + rmsnorm + rope + transpose ----
            es_qkT = ExitStack()
            pool_qkT = es_qkT.enter_context(
                tc.tile_pool(name="pqkT", bufs=1, side="right"))
            qT = pool_qkT.tile([P, NP, L], BF16, tag="qT")
            kT = pool_qkT.tile([P, NP, L], BF16, tag="kT")

            for part in range(3):  # 0=q, 1=k, 2=v
                for tt in range(TT):
                    ts = slice(tt * P, (tt + 1) * P)
                    psw = pool_ps1024.tile([P, 1024], F32, tag="ps1024", name="psw")
                    ps = psw[:, 0:512]
                    for kd in range(KD):
                        nc.tensor.matmul(
                            ps, xT[:, kd, ts],
                            w1sb[:, kd, part * 512:(part + 1) * 512],
                            start=(kd == 0), stop=(kd == KD - 1))
                    if part == 2:  # v
                        nc.scalar.copy(
                            v_sb[:, tt, :, 0:DH],
                            ps.rearrange("p (h d) -> p h d", d=DH))
                        continue
                    # rmsnorm stats
                    sq = pool_w512.tile([P, 512], F32, tag="w512f")
                    nc.scalar.activation(sq, ps, AF.Square)
                    ss = pool_small.tile([P, HL], F32, tag="ss")
                    nc.vector.tensor_reduce(
                        ss, sq.rearrange("p (h d) -> p h d", d=DH),
                        axis=AX.X, op=ALU.add)
                    sd = pool_small.tile([P, HL], F32, tag="sd")
                    nc.scalar.activation(sd, ss, AF.Sqrt, scale=1.0 / DH,
                                         bias=epsc)
                    rs = pool_small.tile([P, HL], F32, tag="rs")
                    nc.vector.reciprocal(rs, sd)
                    rs16 = pool_small.tile([P, HL], BF16, tag="rs16")
                    nc.vector.tensor_copy(rs16, rs)
                    # rope (pe pre-folded with q/k head scales on host)
                    pet = pe_sb[:, part, tt]
                    qs = pool_w512.tile([P, 512], BF16, tag="w512b")
                    nc.scalar.copy(qs, ps)
                    qs4 = qs.rearrange("p (h d t) -> p h d t", d=DH // 2, t=2)
                    qe = qs4[:, :, :, 0].unsqueeze(-1).broadcast_to([P, HL, DH // 2, 2])
                    qo = qs4[:, :, :, 1].unsqueeze(-1).broadcast_to([P, HL, DH // 2, 2])
                    p0 = (pet[:, 0, :].rearrange("p (d t) -> p d t", t=2)
                          .unsqueeze(1).broadcast_to([P, HL, DH // 2, 2]))
                    p1 = (pet[:, 1, :].rearrange("p (d t) -> p d t", t=2)
                          .unsqueeze(1).broadcast_to([P, HL, DH // 2, 2]))
                    t1 = pool_w512.tile([P, 512], BF16, tag="w512b")
                    t2 = pool_w512.tile([P, 512], BF16, tag="w512b")
                    t1v = t1.rearrange("p (h d t) -> p h d t", d=DH // 2, t=2)
                    t2v = t2.rearrange("p (h d t) -> p h d t", d=DH // 2, t=2)
                    nc.gpsimd.tensor_tensor(out=t1v, in0=qe, in1=p0, op=ALU.mult)
                    nc.gpsimd.tensor_tensor(out=t2v, in0=qo, in1=p1, op=ALU.mult)
                    nc.gpsimd.tensor_tensor(out=t1, in0=t1, in1=t2, op=ALU.add)
                    rq = pool_w512.tile([P, 512], BF16, tag="w512b")
                    rqv = rq.rearrange("p (h d) -> p h d", d=DH)
                    rsb = rs16.unsqueeze(-1).broadcast_to([P, HL, DH])
                    nc.vector.tensor_tensor(
                        out=rqv, in0=t1.rearrange("p (h d) -> p h d", d=DH),
                        in1=rsb, op=ALU.mult)
                    # transpose head pairs -> qT/kT [128, pair, tok]
                    dst = qT if part == 0 else kT
                    pst = pool_psb.tile([P, 512], BF16, tag="ps512b")
                    for pr in range(NP):
                        nc.tensor.transpose(
                            pst[:, pr * P:(pr + 1) * P],
                            rq[:, pr * P:(pr + 1) * P], ident)
                    nc.vector.tensor_copy(
                        dst[:, :, ts], pst.rearrange("p (j t) -> p j t", j=NP))
            es_w1.close()

            # ---- Phase B1: linear1 mlp + gelu -> SBUF ----
            for ft in range(FT):
                for tm in range(2):
                    ps = pool_ps1024.tile([P, 1024], F32, tag="ps1024")
                    for tn in range(2):
                        off = tm * 1024 + tn * 512
                        for kd in range(KD):
                            nc.tensor.matmul(
                                ps[:, tn * 512:(tn + 1) * 512],
                                w1m[:, kd, ft * P:(ft + 1) * P],
                                xT[:, kd, off:off + 512],
                                start=(kd == 0), stop=(kd == KD - 1))
                    nc.scalar.activation(
                        mlp_sb[:, ft, tm * 1024:(tm + 1) * 1024], ps,
                        AF.Gelu_apprx_tanh)
            es_xT.close()

            # ---- Phase C: attention ----
            es_attn = ExitStack()
            pool_attn = es_attn.enter_context(
                tc.tile_pool(name="pattn", bufs=1))
            attnT = pool_attn.tile([P, NP, L], BF16, tag="attnT")
            w2sb = pool_attn.tile([P, KT2, DIM], BF16, tag="w2")
            nc.gpsimd.dma_start(w2sb, w2[:, :, :])
            es_c = ExitStack()
            pool_ex = es_c.enter_context(tc.tile_pool(name="pex", bufs=4))
            for ph in range(NP):
                hA, hB = 2 * ph, 2 * ph + 1
                for qc in range(4):
                    qs_ = slice(qc * 512, (qc + 1) * 512)
                    pa = [pool_pacc.tile([P, 512], F32, tag="pacc",
                                         name=f"pa{i}") for i in range(2)]
                    for kt in range(16):
                        kts = slice(kt * P, (kt + 1) * P)
                        pss = pool_ps1024.tile([P, 1024], F32, tag="ps1024")
                        for i, bp in enumerate((0, DH)):
                            nc.tensor.matmul(
                                pss[:, i * 512:(i + 1) * 512],
                                kT[bp:bp + DH, ph, kts],
                                qT[bp:bp + DH, ph, qs_],
                                start=True, stop=True)
                        ex = pool_ex.tile([P, 1024], BF16, tag="ex")
                        nc.scalar.activation(ex, pss, AF.Exp, scale=0.125)
                        for i, h in enumerate((hA, hB)):
                            nc.tensor.matmul(
                                pa[i][0:DH + 1, :], v_sb[:, kt, h, :],
                                ex[:, i * 512:(i + 1) * 512],
                                start=(kt == 0), stop=(kt == 15))
                    bc = pool_ps1024.tile([P, 1024], F32, tag="ps1024")
                    for i, bp in enumerate((0, DH)):
                        nc.scalar.copy(
                            attnT[bp:bp + DH, ph, qs_], pa[i][0:DH, :])
                        rec = pool_w512.tile([P, 512], F32, tag="w512f")
                        nc.vector.reciprocal(
                            rec[DH:DH + 1, :], pa[i][DH:DH + 1, :])
                        recb = pool_w512.tile([P, 512], BF16, tag="w512b")
                        nc.vector.tensor_copy(
                            recb[DH:DH + 1, :], rec[DH:DH + 1, :])
                        nc.tensor.matmul(
                            bc[bp:bp + DH, i * 512:(i + 1) * 512],
                            ones_sb[DH:DH + 1, 0:DH],
                            recb[DH:DH + 1, :], start=True, stop=True)
                        nc.vector.tensor_tensor(
                            out=attnT[bp:bp + DH, ph, qs_],
                            in0=attnT[bp:bp + DH, ph, qs_],
                            in1=bc[bp:bp + DH, i * 512:(i + 1) * 512],
                            op=ALU.mult)
            es_c.close()
            es_qkT.close()

            # ---- Phase E: linear2 (partial; host adds residual + b2) ----
            es_e = ExitStack()
            pool_y = es_e.enter_context(tc.tile_pool(name="py", bufs=2))
            for qt in range(4):
                ys = pool_y.tile([P, 4, DIM], F32, tag="ys")
                for r in range(4):
                    tt = 4 * qt + r
                    ts = slice(tt * P, (tt + 1) * P)
                    ps = pool_ps1024.tile([P, 1024], F32, tag="ps1024")
                    for oc in range(2):
                        pso = ps[:, oc * 512:(oc + 1) * 512]
                        for pr in range(AKT):
                            nc.tensor.matmul(
                                pso, attnT[:, pr, ts],
                                w2sb[:, pr, oc * 512:(oc + 1) * 512],
                                start=(pr == 0), stop=False)
                        for ft in range(FT):
                            nc.tensor.matmul(
                                pso, mlp_sb[:, ft, ts],
                                w2sb[:, AKT + ft, oc * 512:(oc + 1) * 512],
                                start=False, stop=(ft == FT - 1))
                    nc.vector.tensor_copy(ys[:, r, :], ps)
                nc.sync.dma_start(y[:, 4 * qt:4 * qt + 4, :], ys)
            es_e.close()
            es_attn.close()
            es_res.close()

        if repeat == 1:
            body()
        else:
            with tc.For_i(0, repeat, 1):
                body()

    nc.finalize()
    return nc


# ---------------- host side ----------------

_NC_CACHE = {}


def _get_nc(repeat=1):
    if repeat not in _NC_CACHE:
        _NC_CACHE[repeat] = gen_program(repeat)
    return _NC_CACHE[repeat]


def _ptile(a):
    """[L, ...] -> [P, TT*...] per-partition-contiguous layout."""
    rest = a.shape[1:]
    return np.ascontiguousarray(
        a.reshape(TT, P, *rest).transpose(1, 0, *range(2, a.ndim + 1)))


def make_in_maps(x, pe, W1, b1, W2, b2, q_scale, k_scale):
    bf16 = mybir.dt.np(BF16)
    x = np.asarray(x, dtype=np.float32)
    pe = np.asarray(pe, dtype=np.float32)
    W1 = np.asarray(W1, dtype=np.float32)
    W2 = np.asarray(W2, dtype=np.float32)
    q_scale = np.asarray(q_scale, dtype=np.float32)
    k_scale = np.asarray(k_scale, dtype=np.float32)
    assert not np.any(np.asarray(b1)), "kernel assumes b1 == 0"

    pe_r = pe.reshape(L, DH // 2, 2, 2)

    def fold_pe(scale):
        s0 = np.repeat(scale[0::2], 2)  # scale for even input element
        s1 = np.repeat(scale[1::2], 2)
        p0 = pe_r[..., 0].reshape(L, DH) * s0[None, :]
        p1 = pe_r[..., 1].reshape(L, DH) * s1[None, :]
        return np.stack([p0, p1], axis=1)  # [L, 2, DH]

    # [P, 2(q/k), TT, 2, DH]
    pe2 = np.ascontiguousarray(np.stack(
        [_ptile(fold_pe(q_scale)), _ptile(fold_pe(k_scale))],
        axis=1).astype(bf16))

    def wtile(w):
        # [DIM_in, out] -> [P, KD_in, out]
        kd = w.shape[0] // P
        return np.ascontiguousarray(
            w.reshape(kd, P, w.shape[1]).transpose(1, 0, 2).astype(bf16))

    in_maps = []
    for c in range(8):
        b_idx, tp = c // 2, c % 2
        hs = tp * 512
        w1qkv = wtile(np.concatenate(
            [W1[:, hs:hs + 512],
             W1[:, DIM + hs:DIM + hs + 512],
             W1[:, 2 * DIM + hs:2 * DIM + hs + 512]], axis=1))
        w1mlp = wtile(W1[:, 3 * DIM + tp * MLPL:3 * DIM + (tp + 1) * MLPL])
        w2sh = wtile(np.concatenate(
            [W2[hs:hs + 512, :],
             W2[DIM + tp * MLPL:DIM + (tp + 1) * MLPL, :]], axis=0))
        in_maps.append({
            "xb": _ptile(x[b_idx].astype(bf16)),
            "pe2": pe2,
            "w1qkv": w1qkv, "w1mlp": w1mlp, "w2": w2sh,
            "ident": np.eye(P, dtype=bf16),
        })
    return in_maps


def combine_outputs(results, x, b2):
    x = np.asarray(x, dtype=np.float32)
    b2 = np.asarray(b2, dtype=np.float32)
    y = np.empty((B, L, DIM), dtype=np.float32)
    for b_idx in range(B):
        pa = results[2 * b_idx]["y"].transpose(1, 0, 2).reshape(L, DIM)
        pb = results[2 * b_idx + 1]["y"].transpose(1, 0, 2).reshape(L, DIM)
        y[b_idx] = pa + pb + x[b_idx] + b2[None, :]
    return y


def kernel(x, pe, W1, b1, W2, b2, q_scale, k_scale):
    from concourse.bass_utils import run_bass_kernel_spmd
    nc = _get_nc(repeat=1)
    in_maps = make_in_maps(x, pe, W1, b1, W2, b2, q_scale, k_scale)
    res = run_bass_kernel_spmd(nc, in_maps, core_ids=list(range(8)))
    return combine_outputs(res.results, x, b2)


# revision 23
# speedup vs baseline: 2.5897x; 2.5897x over previous
"""Fused ParallelTransformerBlock kernel for 8 Trainium2 NeuronCores.

Sharding: Megatron-style tensor-parallel (2-way over heads + mlp_hidden)
x data-parallel (4-way over batch). Core c handles batch c//2 with
head/mlp shard c%2. Each core computes a partial output of linear2
(no residual, no bias); the host sums the two partials per batch and
adds x + b2 in fp32.

All tensors are bf16 except PSUM accumulation, norm stats and the final
output (fp32). All intermediates stay in SBUF. Every DRAM tensor is
pre-transposed on the host into per-partition-contiguous layout so each
DMA needs only one descriptor per partition (descriptor generation on
the DGE, not bandwidth, dominates scattered transfers). QK^T matmuls
for a head pair run row-tiled (base partitions 0/64) so the two K=64
matmuls overlap on the PE array.
"""
import numpy as np

import concourse.bass as bass
import concourse.tile as tile
from concourse import bacc, mybir

DIM = 1024
L = 2048
B = 4
H = 16
DH = 64
MLP = 3072
EPS_LN = 1e-6
EPS_RMS = 1e-6

P = 128
KD = DIM // P          # 8 k-tiles over model dim
TT = L // P            # 16 token tiles
HL = H // 2            # 8 heads per core
NP = HL // 2           # 4 head pairs
MLPL = MLP // 2        # 1536 mlp columns per core
FT = MLPL // P         # 12 mlp feature tiles
AKT = HL * DH // P     # 4 attn k-tiles into linear2
KT2 = AKT + FT         # 16 linear2 k-tiles

F32 = mybir.dt.float32
BF16 = mybir.dt.bfloat16
AF = mybir.ActivationFunctionType
ALU = mybir.AluOpType
AX = mybir.AxisListType


def gen_program(repeat: int = 1):
    nc = bacc.Bacc("TRN2", target_bir_lowering=False, debug=False, num_devices=8)

    # all inputs pre-transposed to [partition, ...contiguous...] on host
    xb = nc.dram_tensor("xb", (P, TT, DIM), BF16, kind="ExternalInput")
    pe2 = nc.dram_tensor("pe2", (P, 2, TT, 2, DH), BF16, kind="ExternalInput")
    w1qkv = nc.dram_tensor("w1qkv", (P, KD, 3 * HL * DH), BF16, kind="ExternalInput")
    w1mlp = nc.dram_tensor("w1mlp", (P, KD, MLPL), BF16, kind="ExternalInput")
    w2 = nc.dram_tensor("w2", (P, KT2, DIM), BF16, kind="ExternalInput")
    ident_in = nc.dram_tensor("ident", (P, P), BF16, kind="ExternalInput")
    y = nc.dram_tensor("y", (P, TT, DIM), F32, kind="ExternalOutput")

    from contextlib import ExitStack
    with tile.TileContext(nc) as tc, ExitStack() as es:
        pool_const = es.enter_context(tc.tile_pool(name="const", bufs=1))
        pool_w1024 = es.enter_context(tc.tile_pool(name="w1024", bufs=3))
        pool_w512 = es.enter_context(tc.tile_pool(name="w512", bufs=4))
        pool_small = es.enter_context(tc.tile_pool(name="small", bufs=8))
        pool_ps1024 = es.enter_context(tc.tile_pool(name="ps1024", bufs=2, space="PSUM"))
        pool_psb = es.enter_context(tc.tile_pool(name="psb", bufs=2, space="PSUM"))
        pool_pacc = es.enter_context(tc.tile_pool(name="ppacc", bufs=2, space="PSUM"))

        ident = pool_const.tile([P, P], BF16, tag="ident")
        nc.sync.dma_start(ident, ident_in[:, :])
        epsc = pool_const.tile([P, 1], F32, tag="epsc")
        nc.vector.memset(epsc, EPS_LN)
        ones_sb = pool_const.tile([P, DH], BF16, tag="ones_sb")
        nc.vector.memset(ones_sb, 1.0)

        def body():
            es_res = ExitStack()
            pool_res = es_res.enter_context(
                tc.tile_pool(name="pres", bufs=1, side="right"))
            # v laid out per (kt, head) as [tok-in-tile, kt, h, DH+1] with a
            # ones column at the end (accumulates the softmax denominator).
            v_sb = pool_res.tile([P, TT, HL, DH + 1], BF16, tag="v_sb")
            nc.vector.memset(v_sb[:, :, :, DH:DH + 1], 1.0)
            mlp_sb = pool_res.tile([P, FT, L], BF16, tag="mlp_sb")

            es_xT = ExitStack()
            pool_xT = es_xT.enter_context(tc.tile_pool(name="pxT", bufs=1))
            xT = pool_xT.tile([P, KD, L], BF16, tag="xT")
            w1m = pool_xT.tile([P, KD, MLPL], BF16, tag="w1m")
            nc.scalar.dma_start(w1m, w1mlp[:, :, :])

            es_w1 = ExitStack()
            pool_wq = es_w1.enter_context(tc.tile_pool(name="pwq", bufs=1))
            w1sb = pool_wq.tile([P, KD, 3 * HL * DH], BF16, tag="w1sb")
            nc.scalar.dma_start(w1sb, w1qkv[:, :, :])
            pe_sb = pool_wq.tile([P, 2, TT, 2, DH], BF16, tag="pe_sb")
            nc.scalar.dma_start(pe_sb, pe2[:, :, :, :, :])

            # ---- Phase A: LayerNorm + transpose to [dim, tok] ----
            es_xc = ExitStack()
            pool_xc = es_xc.enter_context(tc.tile_pool(name="pxc", bufs=2))
            for qt in range(4):
                xq = pool_xc.tile([P, 4, DIM], BF16, tag="xq")
                nc.sync.dma_start(xq, xb[:, 4 * qt:4 * qt + 4, :])
                for r in range(4):
                    tt = 4 * qt + r
                    ts = slice(tt * P, (tt + 1) * P)
                    xt = xq[:, r, :]
                    st = pool_small.tile([P, 2, 6], F32, tag="st")
                    nc.vector.bn_stats(st[:, 0, :], xt[:, 0:512])
                    nc.vector.bn_stats(st[:, 1, :], xt[:, 512:1024])
                    mv = pool_small.tile([P, 2], F32, tag="mv")
                    nc.vector.bn_aggr(mv, st)
                    std = pool_small.tile([P, 1], F32, tag="std")
                    nc.scalar.activation(std, mv[:, 1:2], AF.Sqrt, bias=epsc)
                    rstd = pool_small.tile([P, 1], F32, tag="rstd")
                    nc.vector.reciprocal(rstd, std)
                    nm = pool_small.tile([P, 1], F32, tag="nm")
                    nc.vector.tensor_scalar(
                        out=nm, in0=mv[:, 0:1], scalar1=rstd, scalar2=-1.0,
                        op0=ALU.mult, op1=ALU.mult)
                    xln = pool_w1024.tile([P, DIM], BF16, tag="w1024b")
                    nc.scalar.activation(xln, xt, AF.Identity,
                                         scale=rstd, bias=nm)
                    for g in range(2):
                        pst = pool_psb.tile([P, 512], BF16, tag="ps512b")
                        for j in range(4):
                            kd = g * 4 + j
                            nc.tensor.transpose(
                                pst[:, j * P:(j + 1) * P],
                                xln[:, kd * P:(kd + 1) * P], ident)
                        nc.vector.tensor_copy(
                            xT[:, g * 4:(g + 1) * 4, ts],
                            pst.rearrange("p (j t) -> p j t", j=4))
            es_xc.close()

            # ---- Phase B2: linear1 qkv + rmsnorm + rope + transpose ----
            es_qkT = ExitStack()
            pool_qkT = es_qkT.enter_context(
                tc.tile_pool(name="pqkT", bufs=1, side="right"))
            qT = pool_qkT.tile([P, NP, L], BF16, tag="qT")
            kT = pool_qkT.tile([P, NP, L], BF16, tag="kT")

            for part in range(3):  # 0=q, 1=k, 2=v
                for tt in range(TT):
                    ts = slice(tt * P, (tt + 1) * P)
                    psw = pool_ps1024.tile([P, 1024], F32, tag="ps1024", name="psw")
                    ps = psw[:, 0:512]
                    for kd in range(KD):
                        nc.tensor.matmul(
                            ps, xT[:, kd, ts],
                            w1sb[:, kd, part * 512:(part + 1) * 512],
                            start=(kd == 0), stop=(kd == KD - 1))
                    if part == 2:  # v
                        nc.scalar.copy(
                            v_sb[:, tt, :, 0:DH],
                            ps.rearrange("p (h d) -> p h d", d=DH))
                        continue
                    # rmsnorm stats
                    sq = pool_w512.tile([P, 512], F32, tag="w512f")
                    nc.scalar.activation(sq, ps, AF.Square)
                    ss = pool_small.tile([P, HL], F32, tag="ss")
                    nc.vector.tensor_reduce(
                        ss, sq.rearrange("p (h d) -> p h d", d=DH),
                        axis=AX.X, op=ALU.add)
                    sd = pool_small.tile([P, HL], F32, tag="sd")
                    nc.scalar.activation(sd, ss, AF.Sqrt, scale=1.0 / DH,
                                         bias=epsc)
                    rs = pool_small.tile([P, HL], F32, tag="rs")
                    nc.vector.reciprocal(rs, sd)
                    rs16 = pool_small.tile([P, HL], BF16, tag="rs16")
                    nc.vector.tensor_copy(rs16, rs)
                    # rope (pe pre-folded with q/k head scales on host)
                    pet = pe_sb[:, part, tt]
                    qs = pool_w512.tile([P, 512], BF16, tag="w512b")
                    nc.scalar.copy(qs, ps)
                    qs4 = qs.rearrange("p (h d t) -> p h d t", d=DH // 2, t=2)
                    qe = qs4[:, :, :, 0].unsqueeze(-1).broadcast_to([P, HL, DH // 2, 2])
                    qo = qs4[:, :, :, 1].unsqueeze(-1).broadcast_to([P, HL, DH // 2, 2])
                    p0 = (pet[:, 0, :].rearrange("p (d t) -> p d t", t=2)
                          .unsqueeze(1).broadcast_to([P, HL, DH // 2, 2]))
                    p1 = (pet[:, 1, :].rearrange("p (d t) -> p d t", t=2)
                          .unsqueeze(1).broadcast_to([P, HL, DH // 2, 2]))
                    t1 = pool_w512.tile([P, 512], BF16, tag="w512b")
                    t2 = pool_w512.tile([P, 512], BF16, tag="w512b")
                    t1v = t1.rearrange("p (h d t) -> p h d t", d=DH // 2, t=2)
                    t2v = t2.rearrange("p (h d t) -> p h d t", d=DH // 2, t=2)
                    nc.gpsimd.tensor_tensor(out=t1v, in0=qe, in1=p0, op=ALU.mult)
                    nc.gpsimd.tensor_tensor(out=t2v, in0=qo, in1=p1, op=ALU.mult)
                    nc.gpsimd.tensor_tensor(out=t1, in0=t1, in1=t2, op=ALU.add)
                    rq = pool_w512.tile([P, 512], BF16, tag="w512b")
                    rqv = rq.rearrange("p (h d) -> p h d", d=DH)
                    rsb = rs16.unsqueeze(-1).broadcast_to([P, HL, DH])
                    nc.vector.tensor_tensor(
                        out=rqv, in0=t1.rearrange("p (h d) -> p h d", d=DH),
                        in1=rsb, op=ALU.mult)
                    # transpose head pairs -> qT/kT [128, pair, tok]
                    dst = qT if part == 0 else kT
                    pst = pool_psb.tile([P, 512], BF16, tag="ps512b")
                    for pr in range(NP):
                        nc.tensor.transpose(
                            pst[:, pr * P:(pr + 1) * P],
                            rq[:, pr * P:(pr + 1) * P], ident)
                    nc.vector.tensor_copy(
                        dst[:, :, ts], pst.rearrange("p (j t) -> p j t", j=NP))
            es_w1.close()

            # ---- Phase B1: linear1 mlp + gelu -> SBUF ----
            for ft in range(FT):
                for tm in range(2):
                    ps = pool_ps1024.tile([P, 1024], F32, tag="ps1024")
                    for tn in range(2):
                        off = tm * 1024 + tn * 512
                        for kd in range(KD):
                            nc.tensor.matmul(
                                ps[:, tn * 512:(tn + 1) * 512],
                                w1m[:, kd, ft * P:(ft + 1) * P],
                                xT[:, kd, off:off + 512],
                                start=(kd == 0), stop=(kd == KD - 1))
                    nc.scalar.activation(
                        mlp_sb[:, ft, tm * 1024:(tm + 1) * 1024], ps,
                        AF.Gelu_apprx_tanh)
            es_xT.close()

            # ---- Phase C: attention ----
            es_attn = ExitStack()
            pool_attn = es_attn.enter_context(
                tc.tile_pool(name="pattn", bufs=1))
            attnT = pool_attn.tile([P, NP, L], BF16, tag="attnT")
            w2sb = pool_attn.tile([P, KT2, DIM], BF16, tag="w2")
            nc.scalar.dma_start(w2sb, w2[:, :, :])
            es_c = ExitStack()
            pool_ex = es_c.enter_context(tc.tile_pool(name="pex", bufs=4))
            for ph in range(NP):
                hA, hB = 2 * ph, 2 * ph + 1
                for qc in range(4):
                    qs_ = slice(qc * 512, (qc + 1) * 512)
                    pa = [pool_pacc.tile([P, 512], F32, tag="pacc",
                                         name=f"pa{i}") for i in range(2)]
                    for kt in range(16):
                        kts = slice(kt * P, (kt + 1) * P)
                        pss = pool_ps1024.tile([P, 1024], F32, tag="ps1024")
                        for i, bp in enumerate((0, DH)):
                            nc.tensor.matmul(
                                pss[:, i * 512:(i + 1) * 512],
                                kT[bp:bp + DH, ph, kts],
                                qT[bp:bp + DH, ph, qs_],
                                start=True, stop=True)
                        ex = pool_ex.tile([P, 1024], BF16, tag="ex")
                        nc.scalar.activation(ex, pss, AF.Exp, scale=0.125)
                        for i, h in enumerate((hA, hB)):
                            nc.tensor.matmul(
                                pa[i][0:DH + 1, :], v_sb[:, kt, h, :],
                                ex[:, i * 512:(i + 1) * 512],
                                start=(kt == 0), stop=(kt == 15))
                    bc = pool_ps1024.tile([P, 1024], F32, tag="ps1024")
                    for i, bp in enumerate((0, DH)):
                        nc.scalar.copy(
                            attnT[bp:bp + DH, ph, qs_], pa[i][0:DH, :])
                        rec = pool_w512.tile([P, 512], F32, tag="w512f")
                        nc.vector.reciprocal(
                            rec[DH:DH + 1, :], pa[i][DH:DH + 1, :])
                        recb = pool_w512.tile([P, 512], BF16, tag="w512b")
                        nc.vector.tensor_copy(
                            recb[DH:DH + 1, :], rec[DH:DH + 1, :])
                        nc.tensor.matmul(
                            bc[bp:bp + DH, i * 512:(i + 1) * 512],
                            ones_sb[DH:DH + 1, 0:DH],
                            recb[DH:DH + 1, :], start=True, stop=True)
                        nc.vector.tensor_tensor(
                            out=attnT[bp:bp + DH, ph, qs_],
                            in0=attnT[bp:bp + DH, ph, qs_],
                            in1=bc[bp:bp + DH, i * 512:(i + 1) * 512],
                            op=ALU.mult)
            es_c.close()
            es_qkT.close()

            # ---- Phase E: linear2 (partial; host adds residual + b2) ----
            es_e = ExitStack()
            pool_y = es_e.enter_context(tc.tile_pool(name="py", bufs=2))
            for qt in range(4):
                ys = pool_y.tile([P, 4, DIM], F32, tag="ys")
                for r in range(4):
                    tt = 4 * qt + r
                    ts = slice(tt * P, (tt + 1) * P)
                    ps = pool_ps1024.tile([P, 1024], F32, tag="ps1024")
                    for oc in range(2):
                        pso = ps[:, oc * 512:(oc + 1) * 512]
                        for pr in range(AKT):
                            nc.tensor.matmul(
                                pso, attnT[:, pr, ts],
                                w2sb[:, pr, oc * 512:(oc + 1) * 512],
                                start=(pr == 0), stop=False)
                        for ft in range(FT):
                            nc.tensor.matmul(
                                pso, mlp_sb[:, ft, ts],
                                w2sb[:, AKT + ft, oc * 512:(oc + 1) * 512],
                                start=False, stop=(ft == FT - 1))
                    nc.vector.tensor_copy(ys[:, r, :], ps)
                nc.sync.dma_start(y[:, 4 * qt:4 * qt + 4, :], ys)
            es_e.close()
            es_attn.close()
            es_res.close()

        if repeat == 1:
            body()
        else:
            with tc.For_i(0, repeat, 1):
                body()

    nc.finalize()
    return nc


# ---------------- host side ----------------

_NC_CACHE = {}


def _get_nc(repeat=1):
    if repeat not in _NC_CACHE:
        _NC_CACHE[repeat] = gen_program(repeat)
    return _NC_CACHE[repeat]


def _ptile(a):
    """[L, ...] -> [P, TT*...] per-partition-contiguous layout."""
    rest = a.shape[1:]
    return np.ascontiguousarray(
        a.reshape(TT, P, *rest).transpose(1, 0, *range(2, a.ndim + 1)))


def make_in_maps(x, pe, W1, b1, W2, b2, q_scale, k_scale):
    bf16 = mybir.dt.np(BF16)
    x = np.asarray(x, dtype=np.float32)
    pe = np.asarray(pe, dtype=np.float32)
    W1 = np.asarray(W1, dtype=np.float32)
    W2 = np.asarray(W2, dtype=np.float32)
    q_scale = np.asarray(q_scale, dtype=np.float32)
    k_scale = np.asarray(k_scale, dtype=np.float32)
    assert not np.any(np.asarray(b1)), "kernel assumes b1 == 0"

    pe_r = pe.reshape(L, DH // 2, 2, 2)

    def fold_pe(scale):
        s0 = np.repeat(scale[0::2], 2)  # scale for even input element
        s1 = np.repeat(scale[1::2], 2)
        p0 = pe_r[..., 0].reshape(L, DH) * s0[None, :]
        p1 = pe_r[..., 1].reshape(L, DH) * s1[None, :]
        return np.stack([p0, p1], axis=1)  # [L, 2, DH]

    # [P, 2(q/k), TT, 2, DH]
    pe2 = np.ascontiguousarray(np.stack(
        [_ptile(fold_pe(q_scale)), _ptile(fold_pe(k_scale))],
        axis=1).astype(bf16))

    def wtile(w):
        # [DIM_in, out] -> [P, KD_in, out]
        kd = w.shape[0] // P
        return np.ascontiguousarray(
            w.reshape(kd, P, w.shape[1]).transpose(1, 0, 2).astype(bf16))

    in_maps = []
    for c in range(8):
        b_idx, tp = c // 2, c % 2
        hs = tp * 512
        w1qkv = wtile(np.concatenate(
            [W1[:, hs:hs + 512],
             W1[:, DIM + hs:DIM + hs + 512],
             W1[:, 2 * DIM + hs:2 * DIM + hs + 512]], axis=1))
        w1mlp = wtile(W1[:, 3 * DIM + tp * MLPL:3 * DIM + (tp + 1) * MLPL])
        w2sh = wtile(np.concatenate(
            [W2[hs:hs + 512, :],
             W2[DIM + tp * MLPL:DIM + (tp + 1) * MLPL, :]], axis=0))
        in_maps.append({
            "xb": _ptile(x[b_idx].astype(bf16)),
            "pe2": pe2,
            "w1qkv": w1qkv, "w1mlp": w1mlp, "w2": w2sh,
            "ident": np.eye(P, dtype=bf16),
        })
    return in_maps


def combine_outputs(results, x, b2):
    x = np.asarray(x, dtype=np.float32)
    b2 = np.asarray(b2, dtype=np.float32)
    y = np.empty((B, L, DIM), dtype=np.float32)
    for b_idx in range(B):
        pa = results[2 * b_idx]["y"].transpose(1, 0, 2).reshape(L, DIM)
        pb = results[2 * b_idx + 1]["y"].transpose(1, 0, 2).reshape(L, DIM)
        y[b_idx] = pa + pb + x[b_idx] + b2[None, :]
    return y


def kernel(x, pe, W1, b1, W2, b2, q_scale, k_scale):
    from concourse.bass_utils import run_bass_kernel_spmd
    nc = _get_nc(repeat=1)
    in_maps = make_in_maps(x, pe, W1, b1, W2, b2, q_scale, k_scale)
    res = run_bass_kernel_spmd(nc, in_maps, core_ids=list(range(8)))
    return combine_outputs(res.results, x, b2)


# revision 25
# speedup vs baseline: 4.1071x; 1.5859x over previous
"""Fused ParallelTransformerBlock kernel for 8 Trainium2 NeuronCores.

Sharding: Megatron-style tensor-parallel (2-way over heads + mlp_hidden)
x data-parallel (4-way over batch). Core c handles batch c//2 with
head/mlp shard c%2. Each core computes a partial output of linear2
(no residual, no bias); the host sums the two partials per batch and
adds x + b2 in fp32.

All tensors are bf16 except PSUM accumulation, norm stats and the final
output (fp32). All intermediates stay in SBUF. Every DRAM tensor is
pre-transposed on the host into per-partition-contiguous layout so each
DMA needs only one descriptor per partition (descriptor generation on
the DGE, not bandwidth, dominates scattered transfers). QK^T matmuls
for a head pair run row-tiled (base partitions 0/64) so the two K=64
matmuls overlap on the PE array.
"""
import numpy as np

import concourse.bass as bass
import concourse.tile as tile
from concourse import bacc, mybir

DIM = 1024
L = 2048
B = 4
H = 16
DH = 64
MLP = 3072
EPS_LN = 1e-6
EPS_RMS = 1e-6

P = 128
KD = DIM // P          # 8 k-tiles over model dim
TT = L // P            # 16 token tiles
HL = H // 2            # 8 heads per core
NP = HL // 2           # 4 head pairs
MLPL = MLP // 2        # 1536 mlp columns per core
FT = MLPL // P         # 12 mlp feature tiles
AKT = HL * DH // P     # 4 attn k-tiles into linear2
KT2 = AKT + FT         # 16 linear2 k-tiles

F32 = mybir.dt.float32
BF16 = mybir.dt.bfloat16
AF = mybir.ActivationFunctionType
ALU = mybir.AluOpType
AX = mybir.AxisListType


def gen_program(repeat: int = 1):
    nc = bacc.Bacc("TRN2", target_bir_lowering=False, debug=False, num_devices=8)

    # all inputs pre-transposed to [partition, ...contiguous...] on host
    xb = nc.dram_tensor("xb", (P, TT, DIM), BF16, kind="ExternalInput")
    pe2 = nc.dram_tensor("pe2", (P, 2, TT, 2, DH), BF16, kind="ExternalInput")
    w1qkv = nc.dram_tensor("w1qkv", (P, KD, 3 * HL * DH), BF16, kind="ExternalInput")
    w1mlp = nc.dram_tensor("w1mlp", (P, KD, MLPL), BF16, kind="ExternalInput")
    w2 = nc.dram_tensor("w2", (P, KT2, DIM), BF16, kind="ExternalInput")
    ident_in = nc.dram_tensor("ident", (P, P), BF16, kind="ExternalInput")
    y = nc.dram_tensor("y", (P, TT, DIM), F32, kind="ExternalOutput")

    from contextlib import ExitStack
    with tile.TileContext(nc) as tc, ExitStack() as es:
        pool_const = es.enter_context(tc.tile_pool(name="const", bufs=1))
        pool_w1024 = es.enter_context(tc.tile_pool(name="w1024", bufs=3))
        pool_w512 = es.enter_context(tc.tile_pool(name="w512", bufs=4))
        pool_small = es.enter_context(tc.tile_pool(name="small", bufs=8))
        pool_ps1024 = es.enter_context(tc.tile_pool(name="ps1024", bufs=2, space="PSUM"))
        pool_psb = es.enter_context(tc.tile_pool(name="psb", bufs=2, space="PSUM"))
        pool_pacc = es.enter_context(tc.tile_pool(name="ppacc", bufs=2, space="PSUM"))

        ident = pool_const.tile([P, P], BF16, tag="ident")
        nc.sync.dma_start(ident, ident_in[:, :])
        epsc = pool_const.tile([P, 1], F32, tag="epsc")
        nc.vector.memset(epsc, EPS_LN)
        ones_sb = pool_const.tile([P, DH], BF16, tag="ones_sb")
        nc.vector.memset(ones_sb, 1.0)

        def body():
            es_res = ExitStack()
            pool_res = es_res.enter_context(
                tc.tile_pool(name="pres", bufs=1, side="right"))
            # v laid out per (kt, head) as [tok-in-tile, kt, h, DH+1] with a
            # ones column at the end (accumulates the softmax denominator).
            v_sb = pool_res.tile([P, TT, HL, DH + 1], BF16, tag="v_sb")
            nc.vector.memset(v_sb[:, :, :, DH:DH + 1], 1.0)
            mlp_sb = pool_res.tile([P, FT, L], BF16, tag="mlp_sb")

            es_xT = ExitStack()
            pool_xT = es_xT.enter_context(tc.tile_pool(name="pxT", bufs=1))
            xT = pool_xT.tile([P, KD, L], BF16, tag="xT")
            w1m = pool_xT.tile([P, KD, MLPL], BF16, tag="w1m")
            nc.scalar.dma_start(w1m, w1mlp[:, :, :])

            es_w1 = ExitStack()
            pool_wq = es_w1.enter_context(tc.tile_pool(name="pwq", bufs=1))
            w1sb = pool_wq.tile([P, KD, 3 * HL * DH], BF16, tag="w1sb")
            nc.scalar.dma_start(w1sb, w1qkv[:, :, :])
            pe_sb = pool_wq.tile([P, 2, TT, 2, DH], BF16, tag="pe_sb")
            nc.scalar.dma_start(pe_sb, pe2[:, :, :, :, :])

            # ---- Phase A: LayerNorm + transpose to [dim, tok] ----
            es_xc = ExitStack()
            pool_xc = es_xc.enter_context(tc.tile_pool(name="pxc", bufs=2))
            for qt in range(4):
                xq = pool_xc.tile([P, 4, DIM], BF16, tag="xq")
                nc.sync.dma_start(xq, xb[:, 4 * qt:4 * qt + 4, :])
                for r in range(4):
                    tt = 4 * qt + r
                    ts = slice(tt * P, (tt + 1) * P)
                    xt = xq[:, r, :]
                    st = pool_small.tile([P, 2, 6], F32, tag="st")
                    nc.vector.bn_stats(st[:, 0, :], xt[:, 0:512])
                    nc.vector.bn_stats(st[:, 1, :], xt[:, 512:1024])
                    mv = pool_small.tile([P, 2], F32, tag="mv")
                    nc.vector.bn_aggr(mv, st)
                    std = pool_small.tile([P, 1], F32, tag="std")
                    nc.scalar.activation(std, mv[:, 1:2], AF.Sqrt, bias=epsc)
                    rstd = pool_small.tile([P, 1], F32, tag="rstd")
                    nc.vector.reciprocal(rstd, std)
                    nm = pool_small.tile([P, 1], F32, tag="nm")
                    nc.vector.tensor_scalar(
                        out=nm, in0=mv[:, 0:1], scalar1=rstd, scalar2=-1.0,
                        op0=ALU.mult, op1=ALU.mult)
                    xln = pool_w1024.tile([P, DIM], BF16, tag="w1024b")
                    nc.scalar.activation(xln, xt, AF.Identity,
                                         scale=rstd, bias=nm)
                    for g in range(2):
                        pst = pool_psb.tile([P, 512], BF16, tag="ps512b")
                        for j in range(4):
                            kd = g * 4 + j
                            nc.tensor.transpose(
                                pst[:, j * P:(j + 1) * P],
                                xln[:, kd * P:(kd + 1) * P], ident)
                        nc.vector.tensor_copy(
                            xT[:, g * 4:(g + 1) * 4, ts],
                            pst.rearrange("p (j t) -> p j t", j=4))
            es_xc.close()

            # ---- Phase B2: linear1 qkv + rmsnorm + rope + transpose ----
            es_qkT = ExitStack()
            pool_qkT = es_qkT.enter_context(
                tc.tile_pool(name="pqkT", bufs=1, side="right"))
            qT = pool_qkT.tile([P, NP, L], BF16, tag="qT")
            kT = pool_qkT.tile([P, NP, L], BF16, tag="kT")

            for tt in range(TT):
                ts = slice(tt * P, (tt + 1) * P)
                psqk = pool_ps1024.tile([P, 1024], F32, tag="ps1024", name="psqk")
                for half in range(2):
                    for kd in range(KD):
                        nc.tensor.matmul(
                            psqk[:, half * 512:(half + 1) * 512],
                            xT[:, kd, ts],
                            w1sb[:, kd, half * 512:(half + 1) * 512],
                            start=(kd == 0), stop=(kd == KD - 1))
                psv = pool_pacc.tile([P, 512], F32, tag="pacc", name="psv")
                for kd in range(KD):
                    nc.tensor.matmul(
                        psv, xT[:, kd, ts], w1sb[:, kd, 1024:1536],
                        start=(kd == 0), stop=(kd == KD - 1))
                nc.scalar.copy(
                    v_sb[:, tt, :, 0:DH],
                    psv.rearrange("p (h d) -> p h d", d=DH))
                for part in range(2):  # 0=q, 1=k
                    ps = psqk[:, part * 512:(part + 1) * 512]
                    # rmsnorm stats
                    sq = pool_w512.tile([P, 512], F32, tag="w512f")
                    nc.scalar.activation(sq, ps, AF.Square)
                    ss = pool_small.tile([P, HL], F32, tag="ss")
                    nc.vector.tensor_reduce(
                        ss, sq.rearrange("p (h d) -> p h d", d=DH),
                        axis=AX.X, op=ALU.add)
                    sd = pool_small.tile([P, HL], F32, tag="sd")
                    nc.scalar.activation(sd, ss, AF.Sqrt, scale=1.0 / DH,
                                         bias=epsc)
                    rs = pool_small.tile([P, HL], F32, tag="rs")
                    nc.vector.reciprocal(rs, sd)
                    rs16 = pool_small.tile([P, HL], BF16, tag="rs16")
                    nc.vector.tensor_copy(rs16, rs)
                    # rope (pe pre-folded with q/k head scales on host)
                    pet = pe_sb[:, part, tt]
                    qs = pool_w512.tile([P, 512], BF16, tag="w512b")
                    nc.scalar.copy(qs, ps)
                    qs4 = qs.rearrange("p (h d t) -> p h d t", d=DH // 2, t=2)
                    qe = qs4[:, :, :, 0].unsqueeze(-1).broadcast_to([P, HL, DH // 2, 2])
                    qo = qs4[:, :, :, 1].unsqueeze(-1).broadcast_to([P, HL, DH // 2, 2])
                    p0 = (pet[:, 0, :].rearrange("p (d t) -> p d t", t=2)
                          .unsqueeze(1).broadcast_to([P, HL, DH // 2, 2]))
                    p1 = (pet[:, 1, :].rearrange("p (d t) -> p d t", t=2)
                          .unsqueeze(1).broadcast_to([P, HL, DH // 2, 2]))
                    t1 = pool_w512.tile([P, 512], BF16, tag="w512b")
                    t2 = pool_w512.tile([P, 512], BF16, tag="w512b")
                    t1v = t1.rearrange("p (h d t) -> p h d t", d=DH // 2, t=2)
                    t2v = t2.rearrange("p (h d t) -> p h d t", d=DH // 2, t=2)
                    nc.gpsimd.tensor_tensor(out=t1v, in0=qe, in1=p0, op=ALU.mult)
                    nc.gpsimd.tensor_tensor(out=t2v, in0=qo, in1=p1, op=ALU.mult)
                    nc.gpsimd.tensor_tensor(out=t1, in0=t1, in1=t2, op=ALU.add)
                    rq = pool_w512.tile([P, 512], BF16, tag="w512b")
                    rqv = rq.rearrange("p (h d) -> p h d", d=DH)
                    rsb = rs16.unsqueeze(-1).broadcast_to([P, HL, DH])
                    nc.vector.tensor_tensor(
                        out=rqv, in0=t1.rearrange("p (h d) -> p h d", d=DH),
                        in1=rsb, op=ALU.mult)
                    # transpose head pairs -> qT/kT [128, pair, tok]
                    dst = qT if part == 0 else kT
                    pst = pool_psb.tile([P, 512], BF16, tag="ps512b")
                    for pr in range(NP):
                        nc.tensor.transpose(
                            pst[:, pr * P:(pr + 1) * P],
                            rq[:, pr * P:(pr + 1) * P], ident)
                    nc.vector.tensor_copy(
                        dst[:, :, ts], pst.rearrange("p (j t) -> p j t", j=NP))
            es_w1.close()

            # ---- Phase B1: linear1 mlp + gelu -> SBUF ----
            for ft in range(FT):
                for tm in range(2):
                    ps = pool_ps1024.tile([P, 1024], F32, tag="ps1024")
                    for tn in range(2):
                        off = tm * 1024 + tn * 512
                        for kd in range(KD):
                            nc.tensor.matmul(
                                ps[:, tn * 512:(tn + 1) * 512],
                                w1m[:, kd, ft * P:(ft + 1) * P],
                                xT[:, kd, off:off + 512],
                                start=(kd == 0), stop=(kd == KD - 1))
                    nc.scalar.activation(
                        mlp_sb[:, ft, tm * 1024:(tm + 1) * 1024], ps,
                        AF.Gelu_apprx_tanh)
            es_xT.close()

            # ---- Phase C: attention ----
            es_attn = ExitStack()
            pool_attn = es_attn.enter_context(
                tc.tile_pool(name="pattn", bufs=1))
            attnT = pool_attn.tile([P, NP, L], BF16, tag="attnT")
            w2sb = pool_attn.tile([P, KT2, DIM], BF16, tag="w2")
            nc.scalar.dma_start(w2sb, w2[:, :, :])
            es_c = ExitStack()
            pool_ex = es_c.enter_context(tc.tile_pool(name="pex", bufs=4))
            for ph in range(NP):
                hA, hB = 2 * ph, 2 * ph + 1
                for qc in range(4):
                    qs_ = slice(qc * 512, (qc + 1) * 512)
                    pa = [pool_pacc.tile([P, 512], F32, tag="pacc",
                                         name=f"pa{i}") for i in range(2)]
                    for kt in range(16):
                        kts = slice(kt * P, (kt + 1) * P)
                        pss = pool_ps1024.tile([P, 1024], F32, tag="ps1024")
                        for i, bp in enumerate((0, DH)):
                            nc.tensor.matmul(
                                pss[:, i * 512:(i + 1) * 512],
                                kT[bp:bp + DH, ph, kts],
                                qT[bp:bp + DH, ph, qs_],
                                start=True, stop=True)
                        ex = pool_ex.tile([P, 1024], BF16, tag="ex")
                        nc.scalar.activation(ex, pss, AF.Exp, scale=0.125)
                        for i, h in enumerate((hA, hB)):
                            nc.tensor.matmul(
                                pa[i][0:DH + 1, :], v_sb[:, kt, h, :],
                                ex[:, i * 512:(i + 1) * 512],
                                start=(kt == 0), stop=(kt == 15))
                    bc = pool_ps1024.tile([P, 1024], F32, tag="ps1024")
                    for i, bp in enumerate((0, DH)):
                        nc.scalar.copy(
                            attnT[bp:bp + DH, ph, qs_], pa[i][0:DH, :])
                        rec = pool_w512.tile([P, 512], F32, tag="w512f")
                        nc.vector.reciprocal(
                            rec[DH:DH + 1, :], pa[i][DH:DH + 1, :])
                        recb = pool_w512.tile([P, 512], BF16, tag="w512b")
                        nc.vector.tensor_copy(
                            recb[DH:DH + 1, :], rec[DH:DH + 1, :])
                        nc.tensor.matmul(
                            bc[bp:bp + DH, i * 512:(i + 1) * 512],
                            ones_sb[DH:DH + 1, 0:DH],
                            recb[DH:DH + 1, :], start=True, stop=True)
                        nc.vector.tensor_tensor(
                            out=attnT[bp:bp + DH, ph, qs_],
                            in0=attnT[bp:bp + DH, ph, qs_],
                            in1=bc[bp:bp + DH, i * 512:(i + 1) * 512],
                            op=ALU.mult)
            es_c.close()
            es_qkT.close()

            # ---- Phase E: linear2 (partial; host adds residual + b2) ----
            es_e = ExitStack()
            pool_y = es_e.enter_context(tc.tile_pool(name="py", bufs=2))
            for qt in range(4):
                ys = pool_y.tile([P, 4, DIM], F32, tag="ys")
                for r in range(4):
                    tt = 4 * qt + r
                    ts = slice(tt * P, (tt + 1) * P)
                    ps = pool_ps1024.tile([P, 1024], F32, tag="ps1024")
                    for oc in range(2):
                        pso = ps[:, oc * 512:(oc + 1) * 512]
                        for pr in range(AKT):
                            nc.tensor.matmul(
                                pso, attnT[:, pr, ts],
                                w2sb[:, pr, oc * 512:(oc + 1) * 512],
                                start=(pr == 0), stop=False)
                        for ft in range(FT):
                            nc.tensor.matmul(
                                pso, mlp_sb[:, ft, ts],
                                w2sb[:, AKT + ft, oc * 512:(oc + 1) * 512],
                                start=False, stop=(ft == FT - 1))
                    nc.vector.tensor_copy(ys[:, r, :], ps)
                nc.sync.dma_start(y[:, 4 * qt:4 * qt + 4, :], ys)
            es_e.close()
            es_attn.close()
            es_res.close()

        if repeat == 1:
            body()
        else:
            with tc.For_i(0, repeat, 1):
                body()

    nc.finalize()
    return nc


# ---------------- host side ----------------

_NC_CACHE = {}


def _get_nc(repeat=1):
    if repeat not in _NC_CACHE:
        _NC_CACHE[repeat] = gen_program(repeat)
    return _NC_CACHE[repeat]


def _ptile(a):
    """[L, ...] -> [P, TT*...] per-partition-contiguous layout."""
    rest = a.shape[1:]
    return np.ascontiguousarray(
        a.reshape(TT, P, *rest).transpose(1, 0, *range(2, a.ndim + 1)))


def make_in_maps(x, pe, W1, b1, W2, b2, q_scale, k_scale):
    bf16 = mybir.dt.np(BF16)
    x = np.asarray(x, dtype=np.float32)
    pe = np.asarray(pe, dtype=np.float32)
    W1 = np.asarray(W1, dtype=np.float32)
    W2 = np.asarray(W2, dtype=np.float32)
    q_scale = np.asarray(q_scale, dtype=np.float32)
    k_scale = np.asarray(k_scale, dtype=np.float32)
    assert not np.any(np.asarray(b1)), "kernel assumes b1 == 0"

    pe_r = pe.reshape(L, DH // 2, 2, 2)

    def fold_pe(scale):
        s0 = np.repeat(scale[0::2], 2)  # scale for even input element
        s1 = np.repeat(scale[1::2], 2)
        p0 = pe_r[..., 0].reshape(L, DH) * s0[None, :]
        p1 = pe_r[..., 1].reshape(L, DH) * s1[None, :]
        return np.stack([p0, p1], axis=1)  # [L, 2, DH]

    # [P, 2(q/k), TT, 2, DH]
    pe2 = np.ascontiguousarray(np.stack(
        [_ptile(fold_pe(q_scale)), _ptile(fold_pe(k_scale))],
        axis=1).astype(bf16))

    def wtile(w):
        # [DIM_in, out] -> [P, KD_in, out]
        kd = w.shape[0] // P
        return np.ascontiguousarray(
            w.reshape(kd, P, w.shape[1]).transpose(1, 0, 2).astype(bf16))

    in_maps = []
    for c in range(8):
        b_idx, tp = c // 2, c % 2
        hs = tp * 512
        w1qkv = wtile(np.concatenate(
            [W1[:, hs:hs + 512],
             W1[:, DIM + hs:DIM + hs + 512],
             W1[:, 2 * DIM + hs:2 * DIM + hs + 512]], axis=1))
        w1mlp = wtile(W1[:, 3 * DIM + tp * MLPL:3 * DIM + (tp + 1) * MLPL])
        w2sh = wtile(np.concatenate(
            [W2[hs:hs + 512, :],
             W2[DIM + tp * MLPL:DIM + (tp + 1) * MLPL, :]], axis=0))
        in_maps.append({
            "xb": _ptile(x[b_idx].astype(bf16)),
            "pe2": pe2,
            "w1qkv": w1qkv, "w1mlp": w1mlp, "w2": w2sh,
            "ident": np.eye(P, dtype=bf16),
        })
    return in_maps


def combine_outputs(results, x, b2):
    x = np.asarray(x, dtype=np.float32)
    b2 = np.asarray(b2, dtype=np.float32)
    y = np.empty((B, L, DIM), dtype=np.float32)
    for b_idx in range(B):
        pa = results[2 * b_idx]["y"].transpose(1, 0, 2).reshape(L, DIM)
        pb = results[2 * b_idx + 1]["y"].transpose(1, 0, 2).reshape(L, DIM)
        y[b_idx] = pa + pb + x[b_idx] + b2[None, :]
    return y


def kernel(x, pe, W1, b1, W2, b2, q_scale, k_scale):
    from concourse.bass_utils import run_bass_kernel_spmd
    nc = _get_nc(repeat=1)
    in_maps = make_in_maps(x, pe, W1, b1, W2, b2, q_scale, k_scale)
    res = run_bass_kernel_spmd(nc, in_maps, core_ids=list(range(8)))
    return combine_outputs(res.results, x, b2)
